# revision 41
# baseline (speedup 1.0000x reference)
# Trainium2 Bass kernel for nn_ActionTransformer3 (transformer + batched GAT).
#
# Sharding (8 cores, SPMD, identical program; per-core variation only via input data):
#  * Tokens: each batch row's 1025 tokens = [cls] + 1024; core c owns a block of
#    128 real tokens per batch row plus a replicated cls column -> 5*(1+128)=645
#    tokens/core ("hT" home layout: transposed [512, 645], dmodel on partitions).
#    Token-local work (encoder, LN, FFN, O-proj, residuals) is fully sharded.
#  * Attention: head-per-core. Per layer, an AllToAll redistributes Q^T,K^T
#    (transposed) and V (token-major) from token-shards to head-shards; each core
#    computes full attention for its head over all 5 batch rows; a second
#    AllToAll returns attention outputs to token shards. Duplicated cls keys
#    (7 copies) are masked by zeroing their V_aug rows (incl. the ones-column
#    used to compute softmax denominators for free).
#  * GAT: 5120 independent 5-node graphs; core c takes timesteps [128c,128c+128)
#    = 640 graphs, laid out graphs-on-partitions; mostly vector/scalar engine
#    work interleaved with the transformer to fill collective/PE gaps.
#
# Precision: fp32 everywhere except attention internals (scores/attV operands,
# A2A wire) and O-proj operands, which use bf16 (fp32 PSUM accumulation).

import math

import numpy as np
import ml_dtypes

# ---------------- constants ----------------
NCORES = 8
L = 4
DM = 512
NH = 8
HD = 64
DFF = 2048
B = 5
SEQ = 129                    # tokens per (batch row, core): col 0 = cls copy
NT = B * SEQ                 # 645 tokens per core
TSG = NCORES * SEQ           # 1032 global keys per batch row
NTOK = 51
MLPD = 256
CLASSES = 7
MC = DM // 128               # 4 dmodel chunks
DC = DFF // 128              # 16 dff chunks
SM_SCALE = 1.0 / 8.0         # 1/sqrt(HD)
EPS = 1e-5

TOKC3 = [(0, 215), (215, 215), (430, 215)]
COLC2 = [(0, 323), (323, 322)]
TQC3 = [(0, 344), (344, 344), (688, 344)]
TS_CHUNKS = [(k * 128, min(128, TSG - k * 128)) for k in range((TSG + 127) // 128)]
VTOKC = [(0, 128), (128, 128), (256, 128), (384, 128), (512, 128), (640, 5)]

# GAT
GH = 3
GHID = 64
GCLS = 5
NGRP = 5
TLOC = 128                   # timesteps per core
NGR = NGRP * TLOC            # 640 graphs per core
GNODE = 5
ALPHA = 0.2
# adjacency (fixed in reference); edges (i, j) grouped by i, j ascending
_ADJ = np.array([[0, 1, 0, 0, 0], [1, 1, 1, 1, 0], [1, 0, 1, 0, 1],
                 [1, 0, 0, 1, 1], [0, 0, 1, 1, 1]], dtype=np.int32)
EDGES = [(i, j) for i in range(5) for j in range(5) if _ADJ[i, j]]
EGRP = []  # (i, start, count) contiguous runs per i
_off = 0
for _i in range(5):
    _cnt = sum(1 for (i, j) in EDGES if i == _i)
    EGRP.append((_i, _off, _cnt))
    _off += _cnt
NEDGE = len(EDGES)  # 14

DEBUG_DUMPS = False  # add per-layer residual-stream dumps as outputs


# ---------------- program builder ----------------

def _build_program():
    import concourse.bass as bass
    import concourse.mybir as mybir
    import concourse.tile as tile
    from concourse import bacc
    from concourse.masks import make_identity

    f32 = mybir.dt.float32
    bf16 = mybir.dt.bfloat16
    AF = mybir.ActivationFunctionType
    OP = mybir.AluOpType
    AX = mybir.AxisListType

    nc = bacc.Bacc(num_devices=NCORES)
    RG = [list(range(NCORES))]

    # ---- external inputs (per-core data marked (pc)) ----
    d_xT = nc.declare_dram_parameter("xT", [NTOK, NT], f32, isOutput=False)          # (pc)
    d_addT = nc.declare_dram_parameter("addT", [128, MC, NT], f32, isOutput=False)   # (pc)
    d_xgT = nc.declare_dram_parameter("xgT", [3, GNODE * NGR], f32, isOutput=False)  # (pc)
    d_wenc = nc.declare_dram_parameter("wenc", [NTOK, DM], f32, isOutput=False)
    d_wqkv = nc.declare_dram_parameter("wqkv", [L, 8, 128, MC, 128], f32, isOutput=False)
    d_wv = nc.declare_dram_parameter("wv", [L, 128, MC, DM], f32, isOutput=False)
    d_wo = nc.declare_dram_parameter("wo", [L, 128, MC, DM], bf16, isOutput=False)
    d_w1 = nc.declare_dram_parameter("w1", [L, 128, MC, DFF], f32, isOutput=False)
    d_w2 = nc.declare_dram_parameter("w2", [L, DC, 128, DM], f32, isOutput=False)
    d_bqk = nc.declare_dram_parameter("bqk", [L, 128, 8], f32, isOutput=False)
    d_bv = nc.declare_dram_parameter("bv", [L, 1, DM], f32, isOutput=False)
    d_bo = nc.declare_dram_parameter("bo", [L, 128, MC], f32, isOutput=False)
    d_b1 = nc.declare_dram_parameter("b1", [L, 128, DC], f32, isOutput=False)
    d_b2 = nc.declare_dram_parameter("b2", [L, 128, MC], f32, isOutput=False)
    d_g1 = nc.declare_dram_parameter("g1", [L, 128, MC], f32, isOutput=False)
    d_be1 = nc.declare_dram_parameter("be1", [L, 128, MC], f32, isOutput=False)
    d_g2 = nc.declare_dram_parameter("g2", [L, 128, MC], f32, isOutput=False)
    d_be2 = nc.declare_dram_parameter("be2", [L, 128, MC], f32, isOutput=False)
    d_wd1 = nc.declare_dram_parameter("wd1", [128, MC, MLPD], f32, isOutput=False)
    d_wd2 = nc.declare_dram_parameter("wd2", [128, 2, CLASSES], f32, isOutput=False)
    d_bd1 = nc.declare_dram_parameter("bd1", [128, 2], f32, isOutput=False)
    d_bd2 = nc.declare_dram_parameter("bd2", [CLASSES, 1], f32, isOutput=False)
    d_wg = nc.declare_dram_parameter("wg", [GH, 3, GHID + 2], f32, isOutput=False)
    d_wgo = nc.declare_dram_parameter("wgo", [GHID, GH, GCLS + 2], f32, isOutput=False)
    d_vmask = nc.declare_dram_parameter("vmask", [128, len(TS_CHUNKS), HD + 1], bf16,
                                        isOutput=False)

    d_dbg = None
    if DEBUG_DUMPS:
        d_dbg = nc.declare_dram_parameter("dbg_h", [L + 1, 128, MC, NT], f32,
                                          isOutput=True)
        d_dbg2 = nc.declare_dram_parameter("dbg_h2", [3, 128, MC, NT], f32,
                                           isOutput=True)
        d_dbg_ao = nc.declare_dram_parameter("dbg_ao", [128, MC, NT], bf16,
                                             isOutput=True)
    d_out_cls = nc.declare_dram_parameter("out_cls", [CLASSES, B], f32, isOutput=True)
    d_out_gat = nc.declare_dram_parameter("out_gat", [TLOC, NGRP, GNODE, GCLS], f32, isOutput=True)

    with tile.TileContext(nc, num_cores=NCORES) as tc:
        import contextlib
        ctx = contextlib.ExitStack()
        with ctx:
            pers = ctx.enter_context(tc.tile_pool(name="pers", bufs=1))
            hpool = ctx.enter_context(tc.tile_pool(name="hpool", bufs=3))
            wpool = ctx.enter_context(tc.tile_pool(name="wpool", bufs=1))
            wqkvp = ctx.enter_context(tc.tile_pool(name="wqkvp", bufs=1))
            biasp = ctx.enter_context(tc.tile_pool(name="biasp", bufs=2))
            qkp = ctx.enter_context(tc.tile_pool(name="qkp", bufs=2))
            vp = ctx.enter_context(tc.tile_pool(name="vp", bufs=2))
            qbp = ctx.enter_context(tc.tile_pool(name="qbp", bufs=2))
            vap = ctx.enter_context(tc.tile_pool(name="vap", bufs=2))
            ptp = ctx.enter_context(tc.tile_pool(name="ptp", bufs=10))
            aop = ctx.enter_context(tc.tile_pool(name="aop", bufs=1))
            f1p = ctx.enter_context(tc.tile_pool(name="f1p", bufs=1))
            smp = ctx.enter_context(tc.tile_pool(name="smp", bufs=2))
            gatp = ctx.enter_context(tc.tile_pool(name="gatp", bufs=1))
            gatsm = ctx.enter_context(tc.tile_pool(name="gatsm", bufs=2))
            pp = ctx.enter_context(tc.tile_pool(name="pp", bufs=6, space="PSUM"))
            gp = ctx.enter_context(tc.tile_pool(name="gp", bufs=2, space="PSUM"))
            dramp = ctx.enter_context(tc.tile_pool(name="dramp", bufs=2, space="DRAM"))

            # ---- singles ----
            ident = pers.tile([128, 128], f32, tag="ident")
            make_identity(nc, ident)
            ones_col = pers.tile([128, 1], f32, tag="ones_col")
            nc.vector.memset(ones_col, 1.0)
            ones_row = pers.tile([1, 128], f32, tag="ones_row")
            nc.vector.memset(ones_row, 1.0)
            eps_sb = pers.tile([1, 1], f32, tag="eps")
            nc.vector.memset(eps_sb, EPS)

            wenc_sb = pers.tile([NTOK, DM], f32, tag="wenc")
            nc.sync.dma_start(out=wenc_sb, in_=d_wenc[:])
            xT_sb = pers.tile([NTOK, NT], f32, tag="xT")
            nc.sync.dma_start(out=xT_sb, in_=d_xT[:])
            wd1_sb = pers.tile([128, MC, MLPD], f32, tag="wd1")
            nc.sync.dma_start(out=wd1_sb, in_=d_wd1[:])
            wd2_sb = pers.tile([128, 2, CLASSES], f32, tag="wd2")
            nc.sync.dma_start(out=wd2_sb, in_=d_wd2[:])
            bd1_sb = pers.tile([128, 2], f32, tag="bd1")
            nc.sync.dma_start(out=bd1_sb, in_=d_bd1[:])
            bd2_sb = pers.tile([CLASSES, 1], f32, tag="bd2")
            nc.sync.dma_start(out=bd2_sb, in_=d_bd2[:])
            wg_sb = pers.tile([3, GH, GHID + 2], f32, tag="wg")
            nc.sync.dma_start(out=wg_sb, in_=d_wg.rearrange("h k f -> k h f"))
            wgo_sb = pers.tile([GHID, GH, GCLS + 2], f32, tag="wgo")
            nc.sync.dma_start(out=wgo_sb, in_=d_wgo[:])
            vmask_sb = pers.tile([128, len(TS_CHUNKS), HD + 1], bf16, tag="vmask")
            nc.sync.dma_start(out=vmask_sb, in_=d_vmask[:])

            # ---------------- GAT (emitted in slices to fill gaps) ----------------
            # persistent graphs-major accumulator for layer-2 input (hcat @ Wgo_aug)
            g2acc = pers.tile([128, NGRP, GNODE, GCLS + 2], f32, tag="g2acc")

            def gat_projA(h):
                """x @ W_aug for head h, transposed chunkwise into graphs-major
                g1 [128 graphs, 5 grp, 5 node, 66]."""
                g1 = gatp.tile([128, NGRP, GNODE, GHID + 2], f32, tag="g1")
                for cc in range(7):  # 3200 = 6*512 + 128
                    c0 = cc * 512
                    cw = min(512, GNODE * NGR - c0)
                    xg = gatp.tile([3, 512], f32, tag="gxg", bufs=2)
                    nc.sync.dma_start(out=xg[:, :cw], in_=d_xgT[:, c0:c0 + cw])
                    ps = gp.tile([GHID + 2, cw], f32, tag="gps")
                    nc.tensor.matmul(ps, lhsT=wg_sb[:, h, :], rhs=xg[:, :cw],
                                     start=True, stop=True)
                    ha = gatp.tile([GHID + 2, 512], f32, tag="ghaug", bufs=2)
                    nc.scalar.copy(ha[:, :cw], ps)
                    for bi in range(cw // 128):
                        n, k = divmod(c0 // 128 + bi, NGRP)
                        ps2 = gp.tile([128, GHID + 2], f32, tag="gps")
                        nc.tensor.transpose(ps2, ha[:, 128 * bi:128 * bi + 128],
                                            ident[:GHID + 2, :GHID + 2])
                        nc.scalar.copy(g1[:, k, n, :], ps2)
                return g1

            def gat_L1(h, g1):
                """GAT attention layer 1 on graphs-major g1, elu, then transpose
                chunkwise and accumulate hcat_h @ Wgo_aug rows into g2acc."""
                # e_ij = lrelu(f1_i + f2_j); att = masked softmax; out = att @ H
                e = gatsm.tile([128, NGRP, NEDGE], f32, tag="ge")
                for m, (i, j) in enumerate(EDGES):
                    nc.vector.tensor_tensor(e[:, :, m:m + 1], g1[:, :, i, GHID:GHID + 1],
                                            g1[:, :, j, GHID + 1:GHID + 2], OP.add)
                etmp2 = gatsm.tile([128, NGRP, NEDGE], f32, tag="ge2")
                nc.vector.tensor_scalar_mul(etmp2, e, ALPHA)
                nc.vector.tensor_tensor(e, e, etmp2, OP.max)
                nc.scalar.activation(e, e, AF.Exp)
                den = gatsm.tile([128, NGRP, GNODE], f32, tag="gden")
                for (i, off, cnt) in EGRP:
                    nc.vector.tensor_reduce(den[:, :, i:i + 1], e[:, :, off:off + cnt],
                                            AX.X, OP.add)
                nc.vector.reciprocal(den, den)
                for (i, off, cnt) in EGRP:
                    nc.vector.tensor_tensor(e[:, :, off:off + cnt], e[:, :, off:off + cnt],
                                            den[:, :, i:i + 1].to_broadcast((128, NGRP, cnt)),
                                            OP.mult)
                out1 = gatp.tile([128, NGRP, GNODE, GHID], f32, tag="gout1")
                tmp = gatp.tile([128, NGRP, GHID], f32, tag="gtmp")
                seen = set()
                for m, (i, j) in enumerate(EDGES):
                    a_bc = e[:, :, m:m + 1].to_broadcast((128, NGRP, GHID))
                    if i not in seen:
                        seen.add(i)
                        nc.vector.tensor_tensor(out1[:, :, i, :], g1[:, :, j, :GHID], a_bc, OP.mult)
                    else:
                        nc.vector.tensor_tensor(tmp, g1[:, :, j, :GHID], a_bc, OP.mult)
                        nc.vector.tensor_tensor(out1[:, :, i, :], out1[:, :, i, :], tmp, OP.add)
                # elu in place, per node slice (keeps the tmp tile small)
                for n in range(GNODE):
                    etmp = gatp.tile([128, NGRP, GHID], f32, tag="gtmp")
                    nc.vector.tensor_scalar_min(etmp, out1[:, :, n, :], 0.0)
                    nc.scalar.activation(etmp, etmp, AF.Exp)
                    nc.scalar.activation(out1[:, :, n, :], out1[:, :, n, :], AF.Relu)
                    nc.vector.scalar_tensor_tensor(out1[:, :, n, :], etmp, -1.0,
                                                   out1[:, :, n, :], OP.add, OP.add)
                # chunkwise: transpose to feature-major, multiply by Wgo_aug rows
                # of this head, transpose product back and accumulate into g2acc
                for cc in range(7):
                    c0 = cc * 512
                    cw = min(512, GNODE * NGR - c0)
                    hc = gatp.tile([GHID, 512], f32, tag="ghcat", bufs=2)
                    for bi in range(cw // 128):
                        n, k = divmod(c0 // 128 + bi, NGRP)
                        ps = gp.tile([GHID, 128], f32, tag="gps")
                        nc.tensor.transpose(ps, out1[:, k, n, :], ident[:, :])
                        nc.scalar.copy(hc[:, 128 * bi:128 * bi + 128], ps)
                    ps2 = gp.tile([GCLS + 2, cw], f32, tag="gps")
                    nc.tensor.matmul(ps2, lhsT=wgo_sb[:, h, :], rhs=hc[:, :cw],
                                     start=True, stop=True)
                    h2t = gatsm.tile([GCLS + 2, 512], f32, tag="gh2t", bufs=1)
                    nc.scalar.copy(h2t[:, :cw], ps2)
                    for bi in range(cw // 128):
                        n, k = divmod(c0 // 128 + bi, NGRP)
                        ps3 = gp.tile([128, GCLS + 2], f32, tag="gps")
                        nc.tensor.transpose(ps3, h2t[:, 128 * bi:128 * bi + 128],
                                            ident[:GCLS + 2, :GCLS + 2])
                        if h == 0:
                            nc.scalar.copy(g2acc[:, k, n, :], ps3)
                        else:
                            nc.vector.tensor_tensor(g2acc[:, k, n, :], g2acc[:, k, n, :],
                                                    ps3, OP.add)

            def gat_L2():
                g2 = g2acc
                e = gatsm.tile([128, NGRP, NEDGE], f32, tag="ge")
                for m, (i, j) in enumerate(EDGES):
                    nc.vector.tensor_tensor(e[:, :, m:m + 1], g2[:, :, i, GCLS:GCLS + 1],
                                            g2[:, :, j, GCLS + 1:GCLS + 2], OP.add)
                etmp2 = gatsm.tile([128, NGRP, NEDGE], f32, tag="ge2")
                nc.vector.tensor_scalar_mul(etmp2, e, ALPHA)
                nc.vector.tensor_tensor(e, e, etmp2, OP.max)
                nc.scalar.activation(e, e, AF.Exp)
                den = gatsm.tile([128, NGRP, GNODE], f32, tag="gden")
                for (i, off, cnt) in EGRP:
                    nc.vector.tensor_reduce(den[:, :, i:i + 1], e[:, :, off:off + cnt],
                                            AX.X, OP.add)
                nc.vector.reciprocal(den, den)
                for (i, off, cnt) in EGRP:
                    nc.vector.tensor_tensor(e[:, :, off:off + cnt], e[:, :, off:off + cnt],
                                            den[:, :, i:i + 1].to_broadcast((128, NGRP, cnt)),
                                            OP.mult)
                out2 = gatsm.tile([128, NGRP, GNODE, GCLS], f32, tag="gout2")
                tmp = gatsm.tile([128, NGRP, GCLS], f32, tag="gtmp2")
                seen = set()
                for m, (i, j) in enumerate(EDGES):
                    a_bc = e[:, :, m:m + 1].to_broadcast((128, NGRP, GCLS))
                    if i not in seen:
                        seen.add(i)
                        nc.vector.tensor_tensor(out2[:, :, i, :], g2[:, :, j, :GCLS], a_bc, OP.mult)
                    else:
                        nc.vector.tensor_tensor(tmp, g2[:, :, j, :GCLS], a_bc, OP.mult)
                        nc.vector.tensor_tensor(out2[:, :, i, :], out2[:, :, i, :], tmp, OP.add)
                etmp = gatsm.tile([128, NGRP, GNODE, GCLS], f32, tag="getmp2")
                nc.vector.tensor_scalar_min(etmp, out2, 0.0)
                nc.scalar.activation(etmp, etmp, AF.Exp)
                nc.scalar.activation(out2, out2, AF.Relu)
                nc.vector.scalar_tensor_tensor(out2, etmp, -1.0, out2, OP.add, OP.add)
                # log_softmax over classes
                ex = gatsm.tile([128, NGRP, GNODE, GCLS], f32, tag="getmp2")
                nc.scalar.activation(ex, out2, AF.Exp)
                lsum = gatsm.tile([128, NGRP, GNODE, 1], f32, tag="glsum")
                nc.vector.tensor_reduce(lsum, ex, AX.X, OP.add)
                nc.scalar.activation(lsum, lsum, AF.Ln)
                nc.vector.tensor_tensor(out2, out2,
                                        lsum.to_broadcast((128, NGRP, GNODE, GCLS)),
                                        OP.subtract)
                nc.sync.dma_start(out=d_out_gat[:], in_=out2)

            # ---------------- transformer helpers ----------------

            def layernorm(h_in, h_out, g_sb, be_sb, mc_g, mc_be):
                """h_out = LN(h_in) * g + be ; h layout [128, MC, NT]."""
                s0 = smp.tile([1, NT], f32, tag="lns0")
                s1 = smp.tile([1, NT], f32, tag="lns1")
                mu = smp.tile([1, NT], f32, tag="lnmu")
                ra = smp.tile([1, NT], f32, tag="lnra")
                rbn = smp.tile([1, NT], f32, tag="lnrb")
                for (c0, cw) in COLC2:
                    ps0 = pp.tile([1, cw], f32, tag="big")
                    ps1 = pp.tile([1, cw], f32, tag="big")
                    for mc in range(MC):
                        nc.tensor.matmul(ps0, lhsT=ones_col, rhs=h_in[:, mc, c0:c0 + cw],
                                         start=(mc == 0), stop=(mc == MC - 1))
                    for mc in range(MC):
                        sqt = smp.tile([128, cw], f32, tag="lnsq")
                        nc.scalar.activation(sqt, h_in[:, mc, c0:c0 + cw], AF.Square)
                        nc.tensor.matmul(ps1, lhsT=ones_col, rhs=sqt,
                                         start=(mc == 0), stop=(mc == MC - 1))
                    nc.scalar.activation(s0[:, c0:c0 + cw], ps0, AF.Copy, scale=1.0 / DM)
                    nc.scalar.activation(s1[:, c0:c0 + cw], ps1, AF.Copy, scale=1.0 / DM)
                nc.scalar.copy(mu, s0)                                # mean
                nc.scalar.activation(s0, s0, AF.Square)               # mean^2
                nc.vector.tensor_tensor(s1, s1, s0, OP.subtract)      # var
                nc.scalar.activation(s1, s1, AF.Sqrt, bias=eps_sb)    # sqrt(var+eps)
                nc.vector.reciprocal(ra, s1)                          # a
                nc.vector.scalar_tensor_tensor(rbn, mu, -1.0, ra,
                                               OP.mult, OP.mult)      # -mean*a
                for (c0, cw) in COLC2:
                    bca = pp.tile([128, cw], f32, tag="big")
                    bcb = pp.tile([128, cw], f32, tag="big")
                    nc.tensor.matmul(bca, lhsT=ones_row, rhs=ra[:, c0:c0 + cw],
                                     start=True, stop=True)
                    nc.tensor.matmul(bcb, lhsT=ones_row, rhs=rbn[:, c0:c0 + cw],
                                     start=True, stop=True)
                    for mc in range(MC):
                        nc.vector.tensor_tensor(h_out[:, mc, c0:c0 + cw],
                                                h_in[:, mc, c0:c0 + cw], bca, OP.mult)
                        nc.vector.tensor_tensor(h_out[:, mc, c0:c0 + cw],
                                                h_out[:, mc, c0:c0 + cw], bcb, OP.add)
                        nc.scalar.activation(h_out[:, mc, c0:c0 + cw], h_out[:, mc, c0:c0 + cw],
                                             AF.Identity, bias=mc_be(be_sb, mc), scale=mc_g(g_sb, mc))

            def mc_col(t, mc):
                return t[:, mc:mc + 1]

            # ---------------- encoder ----------------
            h_cur = hpool.tile([128, MC, NT], f32, tag="h")
            addT_sb = hpool.tile([128, MC, NT], f32, tag="h")
            nc.sync.dma_start(out=addT_sb, in_=d_addT[:])
            for (c0, cw) in COLC2:
                for mc in range(MC):
                    ps = pp.tile([128, cw], f32, tag="big")
                    nc.tensor.matmul(ps, lhsT=wenc_sb[:, mc * 128:mc * 128 + 128],
                                     rhs=xT_sb[:, c0:c0 + cw], start=True, stop=True)
                    nc.vector.tensor_tensor(h_cur[:, mc, c0:c0 + cw], ps,
                                            addT_sb[:, mc, c0:c0 + cw], OP.add)

            if DEBUG_DUMPS:
                nc.sync.dma_start(out=d_dbg[0], in_=h_cur)

            # GAT warmup slice
            gat_g1 = {0: gat_projA(0)}

            # ---------------- layers ----------------
            for l in range(L):
                # --- layer weights / biases ---
                wv_sb = wqkvp.tile([128, MC, DM], f32, tag="wv")
                nc.sync.dma_start(out=wv_sb, in_=d_wv[l])
                wo_sb = wqkvp.tile([128, MC, DM], bf16, tag="wo")
                nc.sync.dma_start(out=wo_sb, in_=d_wo[l])
                w1_sb = wpool.tile([128, MC, DFF], f32, tag="w1")
                nc.sync.dma_start(out=w1_sb, in_=d_w1[l])
                bqk_sb = biasp.tile([128, 8], f32, tag="bqk")
                nc.sync.dma_start(out=bqk_sb, in_=d_bqk[l])
                bv_sb = biasp.tile([1, DM], f32, tag="bv")
                nc.sync.dma_start(out=bv_sb, in_=d_bv[l])
                bo_sb = biasp.tile([128, MC], f32, tag="bo")
                nc.sync.dma_start(out=bo_sb, in_=d_bo[l])
                b1_sb = biasp.tile([128, DC], f32, tag="b1")
                nc.sync.dma_start(out=b1_sb, in_=d_b1[l])
                b2_sb = biasp.tile([128, MC], f32, tag="b2")
                nc.sync.dma_start(out=b2_sb, in_=d_b2[l])
                g1_sb = biasp.tile([128, MC], f32, tag="g1")
                nc.sync.dma_start(out=g1_sb, in_=d_g1[l])
                be1_sb = biasp.tile([128, MC], f32, tag="be1")
                nc.sync.dma_start(out=be1_sb, in_=d_be1[l])
                g2_sb = biasp.tile([128, MC], f32, tag="g2")
                nc.sync.dma_start(out=g2_sb, in_=d_g2[l])
                be2_sb = biasp.tile([128, MC], f32, tag="be2")
                nc.sync.dma_start(out=be2_sb, in_=d_be2[l])

                a2a1_in = dramp.tile([NCORES, 3, HD, NT], bf16, tag="a2a1i")
                a2a1_out = dramp.tile([NCORES, 3, HD, NT], bf16, tag="a2a1o")
                a2a2_in = dramp.tile([NCORES, HD, NT], bf16, tag="a2a2i")
                a2a2_out = dramp.tile([NCORES, HD, NT], bf16, tag="a2a2o")

                # --- Q,K projections (transposed out), pack to A2A input ---
                for oc in range(8):
                    wt = wqkvp.tile([128, MC, 128], f32, tag="wqkvoc")
                    nc.sync.dma_start(out=wt, in_=d_wqkv[l, oc])
                    qk = qkp.tile([128, NT], bf16, tag="qk")
                    for (c0, cw) in COLC2:
                        ps = pp.tile([128, cw], f32, tag="big")
                        for mc in range(MC):
                            nc.tensor.matmul(ps, lhsT=wt[:, mc, :], rhs=h_cur[:, mc, c0:c0 + cw],
                                             start=(mc == 0), stop=(mc == MC - 1))
                        nc.scalar.activation(qk[:, c0:c0 + cw], ps, AF.Identity,
                                             bias=bqk_sb[:, oc:oc + 1])
                    region = 0 if oc < 4 else 1
                    for jj in range(2):
                        head = 2 * (oc % 4) + jj
                        nc.gpsimd.dma_start(out=a2a1_in[head, region],
                                            in_=qk[64 * jj:64 * jj + 64, :])

                # --- V projection (token-major) + bias via ones-row matmul ---
                for (t0, tw) in VTOKC:
                    ps = pp.tile([tw, DM], f32, tag="big")
                    for mc in range(MC):
                        nc.tensor.matmul(ps, lhsT=h_cur[:, mc, t0:t0 + tw],
                                         rhs=wv_sb[:, mc, :], start=(mc == 0), stop=False)
                    nc.tensor.matmul(ps, lhsT=ones_row[0:1, 0:tw], rhs=bv_sb,
                                     start=False, stop=True)
                    v_sb = vp.tile([tw, DM], bf16, tag="v")
                    nc.scalar.copy(v_sb, ps)
                    for head in range(NH):
                        vv = a2a1_in[head, 2].rearrange("a b -> (a b)").rearrange(
                            "(t d) -> t d", d=HD)
                        nc.gpsimd.dma_start(out=vv[t0:t0 + tw, :],
                                            in_=v_sb[:, HD * head:HD * head + HD])

                nc.gpsimd.collective_compute(
                    "AllToAll", mybir.AluOpType.bypass, replica_groups=RG,
                    ins=[a2a1_in[:].opt()], outs=[a2a1_out[:].opt()])

                # GAT filler slices during/after the collective
                if l == 0:
                    gat_L1(0, gat_g1.pop(0)); gat_g1[1] = gat_projA(1)
                elif l == 1:
                    gat_L1(1, gat_g1.pop(1)); gat_g1[2] = gat_projA(2)
                elif l == 2:
                    gat_L1(2, gat_g1.pop(2))
                else:
                    gat_L2()

                # --- attention for my head, per batch row ---
                for b in range(B):
                    att_b = aop.tile([HD, TSG], bf16, tag="att_b", bufs=2)
                    qtb = qbp.tile([HD, TSG], bf16, tag="qtb")
                    ktb = qbp.tile([HD, TSG], bf16, tag="ktb")
                    for r in range(NCORES):
                        nc.gpsimd.dma_start(out=qtb[:, SEQ * r:SEQ * r + SEQ],
                                            in_=a2a1_out[r, 0][:, SEQ * b:SEQ * b + SEQ])
                        nc.gpsimd.dma_start(out=ktb[:, SEQ * r:SEQ * r + SEQ],
                                            in_=a2a1_out[r, 1][:, SEQ * b:SEQ * b + SEQ])
                    vaug = vap.tile([128, len(TS_CHUNKS), HD + 1], bf16, tag="vaug")
                    nc.vector.memset(vaug, 0.0)
                    nc.vector.memset(vaug[:, :, HD:HD + 1], 1.0)
                    for r in range(NCORES):
                        vv = a2a1_out[r, 2].rearrange("a b -> (a b)").rearrange(
                            "(t d) -> t d", d=HD)
                        g0 = SEQ * r
                        k0, o0 = divmod(g0, 128)
                        lenA = min(128 - o0, SEQ)
                        nc.gpsimd.dma_start(out=vaug[o0:o0 + lenA, k0, 0:HD],
                                            in_=vv[SEQ * b:SEQ * b + lenA, :])
                        if lenA < SEQ:
                            nc.gpsimd.dma_start(out=vaug[0:SEQ - lenA, k0 + 1, 0:HD],
                                                in_=vv[SEQ * b + lenA:SEQ * b + SEQ, :])
                    # mask duplicated cls keys + tail garbage (incl. ones column)
                    nc.vector.tensor_tensor(vaug, vaug, vmask_sb, OP.mult)
                    for (q0, qw) in TQC3:
                        pts = []
                        for (s0, sw) in TS_CHUNKS:
                            st = pp.tile([sw, qw], f32, tag="big")
                            nc.tensor.matmul(st, lhsT=ktb[:, s0:s0 + sw],
                                             rhs=qtb[:, q0:q0 + qw], start=True, stop=True)
                            pt = ptp.tile([128, qw], bf16, tag="pt")
                            nc.scalar.activation(pt[:sw, :], st, AF.Exp, scale=SM_SCALE)
                            pts.append(pt)
                        av = pp.tile([HD + 1, qw], f32, tag="big")
                        for k, (s0, sw) in enumerate(TS_CHUNKS):
                            nc.tensor.matmul(av, lhsT=vaug[:sw, k, :], rhs=pts[k][:sw, :],
                                             start=(k == 0), stop=(k == len(TS_CHUNKS) - 1))
                        rec = smp.tile([1, qw], f32, tag="rec")
                        nc.vector.reciprocal(rec, av[HD:HD + 1, :])
                        bc = pp.tile([HD, qw], f32, tag="big")
                        nc.tensor.matmul(bc, lhsT=ones_row[0:1, 0:HD], rhs=rec,
                                         start=True, stop=True)
                        bcs = smp.tile([HD, qw], bf16, tag="bcs")
                        nc.scalar.copy(bcs, bc)
                        nc.vector.tensor_tensor(att_b[:, q0:q0 + qw], av[0:HD, :],
                                                bcs, OP.mult)
                    for r in range(NCORES):
                        nc.gpsimd.dma_start(out=a2a2_in[r][:, SEQ * b:SEQ * b + SEQ],
                                            in_=att_b[:, SEQ * r:SEQ * r + SEQ])

                # --- A2A back to token shards ---
                nc.gpsimd.collective_compute(
                    "AllToAll", mybir.AluOpType.bypass, replica_groups=RG,
                    ins=[a2a2_in[:].opt()], outs=[a2a2_out[:].opt()])
                ao = aop.tile([128, MC, NT], bf16, tag="ao")
                for r in range(NCORES):
                    nc.gpsimd.dma_start(out=ao[64 * (r % 2):64 * (r % 2) + 64, r // 2, :],
                                        in_=a2a2_out[r])

                if DEBUG_DUMPS and l == 0:
                    nc.sync.dma_start(out=d_dbg_ao[:], in_=ao)
                # --- O-projection + bias + residual (in-place into h_cur) ---
                for mc in range(MC):
                    for (t0, tw) in TOKC3:
                        ps = pp.tile([128, tw], f32, tag="big")
                        for kc in range(MC):
                            nc.tensor.matmul(ps, lhsT=wo_sb[:, kc, mc * 128:mc * 128 + 128],
                                             rhs=ao[:, kc, t0:t0 + tw],
                                             start=(kc == 0), stop=(kc == MC - 1))
                        nc.vector.scalar_tensor_tensor(h_cur[:, mc, t0:t0 + tw], ps,
                                                       bo_sb[:, mc:mc + 1],
                                                       h_cur[:, mc, t0:t0 + tw],
                                                       OP.add, OP.add)

                if DEBUG_DUMPS and l == 0:
                    nc.sync.dma_start(out=d_dbg2[0], in_=h_cur)
                # --- LN1 ---
                h1n = hpool.tile([128, MC, NT], f32, tag="h")
                layernorm(h_cur, h1n, g1_sb, be1_sb, mc_col, mc_col)
                if DEBUG_DUMPS and l == 0:
                    nc.sync.dma_start(out=d_dbg2[1], in_=h1n)

                # --- FFN (+ residual into h_cur), then LN2 -> next h ---
                for (t0, tw) in TOKC3:
                    f1t = f1p.tile([128, DC, tw], f32, tag="f1")
                    for dc in range(DC):
                        ps = pp.tile([128, tw], f32, tag="big")
                        for mc in range(MC):
                            nc.tensor.matmul(ps, lhsT=w1_sb[:, mc, dc * 128:dc * 128 + 128],
                                             rhs=h1n[:, mc, t0:t0 + tw],
                                             start=(mc == 0), stop=(mc == MC - 1))
                        nc.scalar.activation(f1t[:, dc, :], ps, AF.Relu,
                                             bias=b1_sb[:, dc:dc + 1])
                    ps_m = [pp.tile([128, tw], f32, tag="big", name=f"psm{mc}")
                            for mc in range(MC)]
                    for dc in range(DC):
                        w2c = wqkvp.tile([128, DM], f32, tag="w2c", bufs=3)
                        nc.sync.dma_start(out=w2c, in_=d_w2[l, dc])
                        for mc in range(MC):
                            nc.tensor.matmul(ps_m[mc], lhsT=w2c[:, mc * 128:mc * 128 + 128],
                                             rhs=f1t[:, dc, :],
                                             start=(dc == 0), stop=(dc == DC - 1))
                    for mc in range(MC):
                        nc.vector.scalar_tensor_tensor(h_cur[:, mc, t0:t0 + tw], ps_m[mc],
                                                       b2_sb[:, mc:mc + 1],
                                                       h1n[:, mc, t0:t0 + tw],
                                                       OP.add, OP.add)
                if DEBUG_DUMPS and l == 0:
                    nc.sync.dma_start(out=d_dbg2[2], in_=h_cur)
                h2n = hpool.tile([128, MC, NT], f32, tag="h")
                layernorm(h_cur, h2n, g2_sb, be2_sb, mc_col, mc_col)
                h_cur = h2n
                if DEBUG_DUMPS:
                    nc.sync.dma_start(out=d_dbg[l + 1], in_=h_cur)

            # ---------------- classification head on cls columns ----------------
            cls_ap = [h_cur[:, mc, :].rearrange("p (b s) -> p b s", b=B)[:, :, 0]
                      for mc in range(MC)]
            out1 = smp.tile([128, 2, B], f32, tag="out1")
            for o in range(2):
                ps = pp.tile([128, B], f32, tag="big")
                for mc in range(MC):
                    nc.tensor.matmul(ps, lhsT=wd1_sb[:, mc, o * 128:o * 128 + 128],
                                     rhs=cls_ap[mc], start=(mc == 0), stop=(mc == MC - 1))
                nc.scalar.activation(out1[:, o, :], ps, AF.Identity, bias=bd1_sb[:, o:o + 1])
            ps = pp.tile([CLASSES, B], f32, tag="big")
            for o in range(2):
                nc.tensor.matmul(ps, lhsT=wd2_sb[:, o, :], rhs=out1[:, o, :],
                                 start=(o == 0), stop=(o == 1))
            out2 = smp.tile([CLASSES, B], f32, tag="out2")
            nc.scalar.activation(out2, ps, AF.Identity, bias=bd2_sb)
            nc.sync.dma_start(out=d_out_cls[:], in_=out2)

    nc.finalize()
    return nc


# ---------------- host-side input prep ----------------

def _prep_inputs(inputs):
    """Build per-core input maps from the full problem inputs."""
    f32 = np.float32
    bf16 = ml_dtypes.bfloat16
    x = np.asarray(inputs["x"], f32)
    W_enc = np.asarray(inputs["W_enc"], f32)
    b_enc = np.asarray(inputs["b_enc"], f32)
    cls_tok = np.asarray(inputs["cls_tok"], f32).reshape(DM)
    pos_emb = np.asarray(inputs["pos_emb"], f32)
    Wq = np.asarray(inputs["Wq"], f32); bq = np.asarray(inputs["bq"], f32)
    Wk = np.asarray(inputs["Wk"], f32); bk = np.asarray(inputs["bk"], f32)
    Wv = np.asarray(inputs["Wv"], f32); bv = np.asarray(inputs["bv"], f32)
    Wo = np.asarray(inputs["Wo"], f32); bo = np.asarray(inputs["bo"], f32)
    W1 = np.asarray(inputs["W1"], f32); b1 = np.asarray(inputs["b1"], f32)
    W2 = np.asarray(inputs["W2"], f32); b2 = np.asarray(inputs["b2"], f32)
    g1 = np.asarray(inputs["g1"], f32); be1 = np.asarray(inputs["be1"], f32)
    g2 = np.asarray(inputs["g2"], f32); be2 = np.asarray(inputs["be2"], f32)
    Wd1 = np.asarray(inputs["Wd1"], f32); bd1 = np.asarray(inputs["bd1"], f32)
    Wd2 = np.asarray(inputs["Wd2"], f32); bd2 = np.asarray(inputs["bd2"], f32)
    Wg = np.asarray(inputs["Wg"], f32); ag = np.asarray(inputs["ag"], f32)
    Wgo = np.asarray(inputs["Wgo"], f32); ago = np.asarray(inputs["ago"], f32)

    # replicated weights
    qkv = np.concatenate([Wq, Wk], axis=2)                       # [L, 512, 1024]
    wqkv = np.ascontiguousarray(
        qkv.reshape(L, MC, 128, 8, 128).transpose(0, 3, 2, 1, 4))  # [L, 12? -> 8 oc]
    wv = np.ascontiguousarray(Wv.reshape(L, MC, 128, DM).transpose(0, 2, 1, 3))
    wo = np.ascontiguousarray(Wo.reshape(L, MC, 128, DM).transpose(0, 2, 1, 3)).astype(bf16)
    w1 = np.ascontiguousarray(W1.reshape(L, MC, 128, DFF).transpose(0, 2, 1, 3))
    w2 = np.ascontiguousarray(W2.reshape(L, DC, 128, DM))
    bqk = np.ascontiguousarray(
        np.concatenate([bq, bk], axis=1).reshape(L, 8, 128).transpose(0, 2, 1))
    bvr = np.ascontiguousarray(bv.reshape(L, 1, DM))
    bo_t = np.ascontiguousarray(bo.reshape(L, MC, 128).transpose(0, 2, 1))
    b1_t = np.ascontiguousarray(b1.reshape(L, DC, 128).transpose(0, 2, 1))
    b2_t = np.ascontiguousarray(b2.reshape(L, MC, 128).transpose(0, 2, 1))
    g1_t = np.ascontiguousarray(g1.reshape(L, MC, 128).transpose(0, 2, 1))
    be1_t = np.ascontiguousarray(be1.reshape(L, MC, 128).transpose(0, 2, 1))
    g2_t = np.ascontiguousarray(g2.reshape(L, MC, 128).transpose(0, 2, 1))
    be2_t = np.ascontiguousarray(be2.reshape(L, MC, 128).transpose(0, 2, 1))
    wd1 = np.ascontiguousarray(Wd1.reshape(MC, 128, MLPD).transpose(1, 0, 2))
    wd2 = np.ascontiguousarray(Wd2.reshape(2, 128, CLASSES).transpose(1, 0, 2))
    bd1_t = np.ascontiguousarray(bd1.reshape(2, 128).T)
    bd2_t = np.ascontiguousarray(bd2.reshape(CLASSES, 1))
    # GAT augmented weights
    wg = np.stack([np.concatenate([Wg[h], (Wg[h] @ ag[h, :GHID])[:, None],
                                   (Wg[h] @ ag[h, GHID:])[:, None]], axis=1)
                   for h in range(GH)], axis=0)                   # [3, 3, 66]
    wgo_full = np.concatenate([Wgo, (Wgo @ ago[:GCLS])[:, None],
                               (Wgo @ ago[GCLS:])[:, None]], axis=1)  # [192, 7]
    wgo = np.ascontiguousarray(
        wgo_full.reshape(GH, GHID, GCLS + 2).transpose(1, 0, 2))      # [64, 3, 7]

    # V_aug mask: zero duplicated cls keys (cores 1..7) and the ragged tail
    nts = (TSG + 127) // 128
    vmask = np.ones((128, nts, HD + 1), np.float32)
    for r in range(1, NCORES):
        k0, o0 = divmod(SEQ * r, 128)
        vmask[o0, k0, :] = 0.0
    for g in range(TSG, nts * 128):
        k0, o0 = divmod(g, 128)
        vmask[o0, k0, :] = 0.0
    vmask = vmask.astype(bf16)

    shared = dict(
        wenc=W_enc, wqkv=wqkv, wv=wv, wo=wo, w1=w1, w2=w2, bqk=bqk, bv=bvr,
        bo=bo_t, b1=b1_t, b2=b2_t, g1=g1_t, be1=be1_t, g2=g2_t, be2=be2_t,
        wd1=wd1, wd2=wd2, bd1=bd1_t, bd2=bd2_t, wg=wg, wgo=wgo, vmask=vmask)

    in_maps = []
    for c in range(NCORES):
        xs = np.zeros((B, SEQ, NTOK), f32)
        xs[:, 1:, :] = x[:, 128 * c:128 * c + 128, :]
        xT = np.ascontiguousarray(xs.reshape(NT, NTOK).T)
        add = np.empty((B, SEQ, DM), f32)
        add[:, 0, :] = cls_tok + pos_emb[0] + b_enc
        add[:, 1:, :] = pos_emb[1 + 128 * c: 129 + 128 * c][None] + b_enc
        addT = np.ascontiguousarray(
            add.reshape(NT, DM).T.reshape(MC, 128, NT).transpose(1, 0, 2))
        # GAT input: col = n*640 + grp*128 + t_local ; rows = 3 feats
        xg = x[:, 128 * c:128 * c + 128, :15].reshape(B, TLOC, NGRP, 3)
        xgT = np.ascontiguousarray(xg.transpose(3, 0, 2, 1).reshape(3, GNODE * NGR))
        m = dict(shared)
        m.update(xT=xT, addT=addT, xgT=xgT)
        in_maps.append(m)
    return in_maps


_CACHE = {}


def kernel(**inputs):
    from concourse.bass_utils import run_bass_kernel_spmd

    if "nc" not in _CACHE:
        _CACHE["nc"] = _build_program()
    nc = _CACHE["nc"]
    in_maps = _prep_inputs(inputs)
    res = run_bass_kernel_spmd(nc, in_maps, list(range(NCORES)))
    results = res.results
    out = np.ascontiguousarray(results[0]["out_cls"].T.astype(np.float32))
    gat = np.stack([results[c]["out_gat"] for c in range(NCORES)], axis=0)
    gat = np.ascontiguousarray(gat.reshape(NCORES * TLOC, NGRP, GNODE, GCLS)
                               .astype(np.float32))
    return out, gat
